# revision 40
# baseline (speedup 1.0000x reference)
"""LoRA cross-attention kernel for 8 Trainium2 NeuronCores (axon-tunneled).

The end-to-end wall time is dominated by host<->device transfer over the axon
tunnel (~55 MB/s), so the design minimizes bytes moved:

  - x / context are uploaded int8 (per-feature absmax scales), pre-transposed
    and *sequence-sharded* (512 tokens per core, no duplication), then
    all-gathered on device across each batch's 4-core group and dequantized
    to bf16 in SBUF.
  - Base projection weights (Wq,Wk,Wv,Wo slices) are uploaded int8 (per
    output channel scales), split across the two batch groups and
    all-gathered across core pairs {d, d+4}. The integer weights multiply
    directly (exact in bf16); the channel scale is applied to the projection
    PSUM output (LoRA up-proj factors are pre-divided by the scale on host
    so base+LoRA accumulate in one PSUM group).
  - to_out partials are reduce-scattered on device (f32); each core packs its
    [256, 2048] slice of out^T into 12 bits/element (high byte + packed low
    nibbles + per-row f32 absmax) in a SINGLE uint8 output tensor — one
    output array minimizes per-fetch round-trip latency, which dominates the
    download cost on this tunnel.
  - Donated PJRT output buffers reuse the previous call's device-resident
    outputs (on-device jnp.zeros on the first call) — output buffers are
    never uploaded; the shard_map jit is built once and cached.

Per-call tunnel traffic: ~13 MB up + 6 MB down (f32 baseline: ~145 + 64 MB),
each direction a single global array (8 per-shard transfers). Quantization
keeps rel err ~1.7e-2 (< 2e-2 gate); measured in numpy ahead of time against
the exact device dataflow.

Sharding: core d handles batch b=d//4, head slice h in [4*(d%4), 4*(d%4)+4)
(inner slice of 256 = 4*64). Device dataflow (bf16 matmuls, f32 PSUM):
  xT,cT   [128,8,2048]  x^T / context^T dequantized from gathered int8
  lowT    [32,2048]     [Ak;Av]-low rank projections of context
  qT,kT   [128,2,2048]  q^T, k^T (inner on partitions); kT includes LoRA
  v       [128,16,4,65] v in [m, head, dh+1] layout, col 64 = ones
  simT    psum[m,2,512] per head pair via row-tiled (tile_position) matmuls
  e       exp(SCALE*simT) on ScalarE -> bf16
  attn@v  lhsT=v_aug[m,65], rhs=e -> psum[65,n]: rows 0:64 out^T, row 64 denom
  norm    recip(denom) broadcast via K=1 matmul, DVE multiply
  to_out  WoT.T @ outT -> partial final^T [1024,2048] f32 -> HBM -> RS
"""

import numpy as np
import ml_dtypes

import concourse.bass as bass
import concourse.mybir as mybir
import concourse.tile as tile

BF16 = mybir.dt.bfloat16
F32 = mybir.dt.float32
I8 = mybir.dt.int8
AF = mybir.ActivationFunctionType

N = 2048      # query length
M = 2048      # context length
D = 1024      # model dim
IS = 256      # inner slice per core (4 heads * 64)
DH = 64
NHEADS = 4    # heads per core
SCALE = DH ** -0.5
NB = 512      # n-block (free dim tile)
N_NB = N // NB
N_MB = M // 128
SS = 512      # sequence shard per core (N/4)
REG = SS * D  # elements per blob region (one int8 [1024, 512] slab)
# byte offsets of the small side tensors appended to the int8 blob
AB_OFF = 3 * REG                 # abT   [1024, 32] bf16 (65536 B)
BK_OFF = AB_OFF + 65536          # bkT0  [32, 256] bf16 (16384 B)
BV_OFF = BK_OFF + 16384          # b0vT  [32, 256] bf16 (16384 B)
SCL_OFF = BV_OFF + 16384         # scl   [3840] f32 (15360 B)
BLOB_BYTES = SCL_OFF + 15360

G4 = [[0, 1, 2, 3], [4, 5, 6, 7]]          # batch groups
GP = [[0, 4], [1, 5], [2, 6], [3, 7]]      # weight-dedup pairs

_CACHE = {}


def _emit(tc, nc, d):
    from contextlib import ExitStack
    ctx = ExitStack()
    P1 = ctx.enter_context(tc.tile_pool(name="persist", bufs=1))
    WK = ctx.enter_context(tc.tile_pool(name="work", bufs=8))
    PS = ctx.enter_context(tc.tile_pool(name="psum", bufs=2, space="PSUM"))
    PO = ctx.enter_context(tc.tile_pool(name="psum_o", bufs=2, space="PSUM"))
    PJ = ctx.enter_context(tc.tile_pool(name="psum_j", bufs=2, space="PSUM"))

    xT = P1.tile([128, 8, N], BF16)
    cT = P1.tile([128, 8, M], BF16)
    wq = P1.tile([128, 8, IS], BF16)
    wk = P1.tile([128, 8, IS], BF16)
    wv = P1.tile([128, 8, IS], BF16)
    ab = P1.tile([128, 8, 32], BF16)
    bk = P1.tile([32, IS], BF16)
    bv = P1.tile([32, IS], BF16)
    wo = P1.tile([128, 2, D], BF16)
    qT = P1.tile([128, 2, N], BF16)
    kT = P1.tile([128, 2, M], BF16)
    vA = P1.tile([128, N_MB, NHEADS, DH + 1], BF16)
    oT = P1.tile([128, 2, N], BF16)
    low = P1.tile([32, M], BF16)
    ones64 = P1.tile([1, DH], BF16)
    ident = P1.tile([64, 64], BF16)
    sq_sb = P1.tile([128, 2], F32)
    sk_sb = P1.tile([128, 2], F32)
    so_sb = P1.tile([128, 8], F32)
    sx_sb = P1.tile([128, 8], F32)
    sc_sb = P1.tile([128, 8], F32)
    sv_row = P1.tile([1, IS], F32)
    svb = P1.tile([128, NHEADS, DH], F32)
    ones1 = P1.tile([1, 128], F32)

    # ---- bounce blob regions to internal DRAM and gather across cores ----
    nc.sync.dma_start(d["cb8"][:], d["blob8"][0:REG].rearrange(
        "(p j) -> p j", j=SS))
    nc.sync.dma_start(d["wb8"][:], d["blob8"][REG:2 * REG].rearrange(
        "(w r c) -> w r c", r=256, c=D))
    nc.sync.dma_start(d["xb8"][:], d["blob8"][2 * REG:3 * REG].rearrange(
        "(p j) -> p j", j=SS))
    nc.gpsimd.collective_compute(
        "AllGather", mybir.AluOpType.bypass, replica_groups=G4,
        ins=[d["cb8"][:]], outs=[d["cg8"][:]])
    nc.gpsimd.collective_compute(
        "AllGather", mybir.AluOpType.bypass, replica_groups=GP,
        ins=[d["wb8"][:]], outs=[d["wg8"][:]])
    nc.gpsimd.collective_compute(
        "AllGather", mybir.AluOpType.bypass, replica_groups=G4,
        ins=[d["xb8"][:]], outs=[d["xg8"][:]])

    # ---- scale loads (f32 bytes appended to blob) + sv broadcast ----
    def scl_ap(lo, hi):
        return d["blob8"][SCL_OFF + 4 * lo:SCL_OFF + 4 * hi].bitcast(F32)

    nc.sync.dma_start(sq_sb[:], scl_ap(0, 256).rearrange("(a p) -> p a", p=128))
    nc.sync.dma_start(sk_sb[:], scl_ap(256, 512).rearrange("(a p) -> p a", p=128))
    nc.sync.dma_start(sv_row[:], scl_ap(512, 768).rearrange("(a p) -> a p", a=1))
    nc.sync.dma_start(so_sb[:], scl_ap(768, 1792).rearrange("(a p) -> p a", p=128))
    nc.sync.dma_start(sx_sb[:], scl_ap(1792, 2816).rearrange("(a p) -> p a", p=128))
    nc.sync.dma_start(sc_sb[:], scl_ap(2816, 3840).rearrange("(a p) -> p a", p=128))
    nc.gpsimd.memset(ones1[:], 1.0)
    pbb = PJ.tile([128, NB], F32, tag="pj")
    nc.tensor.matmul(pbb[:, 0:IS], ones1[:], sv_row[:], start=True, stop=True)
    nc.vector.tensor_copy(svb[:], pbb[:, 0:IS].rearrange(
        "p (h e) -> p h e", h=NHEADS))

    # ---- SBUF loads: dequantize gathered int8 into bf16 tiles ----
    # (scoped pool so its SBUF is released before the packing epilogue)
    ld_ctx = tc.tile_pool(name="load8", bufs=4)
    LD = ld_ctx.__enter__()

    def load_ct(kb):
        for g4 in range(4):
            t8 = LD.tile([128, SS], I8, tag="i8")
            nc.sync.dma_start(t8[:], d["cg8"][g4, kb * 128:(kb + 1) * 128, :])
            nc.vector.tensor_scalar_mul(
                cT[:, kb, SS * g4:SS * (g4 + 1)], t8[:], sc_sb[:, kb:kb + 1])

    def load_xt(kb):
        for g4 in range(4):
            t8 = LD.tile([128, SS], I8, tag="i8")
            nc.sync.dma_start(t8[:], d["xg8"][g4, kb * 128:(kb + 1) * 128, :])
            nc.vector.tensor_scalar_mul(
                xT[:, kb, SS * g4:SS * (g4 + 1)], t8[:], sx_sb[:, kb:kb + 1])

    for kb in range(8):
        load_ct(kb)
    nc.sync.dma_start(ab[:], d["blob8"][AB_OFF:AB_OFF + 65536].bitcast(
        BF16).rearrange("(ko ki r) -> ki ko r", ki=128, r=32))
    w8k = LD.tile([128, 8, IS], I8, tag="w8")
    nc.sync.dma_start(w8k[:], d["wg8"][1].rearrange(
        "(p a) (c i) -> p (a c) i", a=2, i=IS))
    nc.vector.tensor_copy(wk[:], w8k[:])
    nc.sync.dma_start(bk[:], d["blob8"][BK_OFF:BK_OFF + 16384].bitcast(
        BF16).rearrange("(a b) -> a b", b=IS))
    for kb in range(8):
        load_xt(kb)
    w8q = LD.tile([128, 8, IS], I8, tag="w8")
    nc.sync.dma_start(w8q[:], d["wg8"][0].rearrange(
        "(p a) (c i) -> p (a c) i", a=2, i=IS))
    nc.vector.tensor_copy(wq[:], w8q[:])
    w8v = LD.tile([128, 8, IS], I8, tag="w8")
    nc.sync.dma_start(w8v[:], d["wg8"][2].rearrange(
        "(p a) (c i) -> p (a c) i", a=2, i=IS))
    nc.vector.tensor_copy(wv[:], w8v[:])
    nc.sync.dma_start(bv[:], d["blob8"][BV_OFF:BV_OFF + 16384].bitcast(
        BF16).rearrange("(a b) -> a b", b=IS))
    w8o = LD.tile([128, 2, D], I8, tag="w8")
    nc.sync.dma_start(w8o[:], d["wg8"][3].rearrange("(p a) dd -> p a dd", a=2))
    nc.vector.tensor_copy(wo[:], w8o[:])
    ld_ctx.__exit__(None, None, None)
    nc.gpsimd.memset(ones64[:], 1.0)
    nc.gpsimd.memset(vA[:, :, :, DH], 1.0)
    from concourse.masks import make_identity
    make_identity(nc, ident[:])

    # ---- lowT = [Ak|Av]^T-proj of context: [32, M] ----
    for nb in range(M // NB):
        pl = PJ.tile([128, NB], F32, tag="pj")
        for kb in range(8):
            nc.tensor.matmul(pl[0:32, :], ab[:, kb, :], cT[:, kb, bass.ts(nb, NB)],
                             start=(kb == 0), stop=(kb == 7))
        nc.vector.tensor_copy(low[:, bass.ts(nb, NB)], pl[0:32, :])

    def proj_q_chunk(ib, nb):
        pq = PJ.tile([128, NB], F32, tag="pj")
        for kb in range(8):
            nc.tensor.matmul(pq[:, :], wq[:, kb, bass.ts(ib, 128)],
                             xT[:, kb, bass.ts(nb, NB)],
                             start=(kb == 0), stop=(kb == 7))
        nc.vector.tensor_scalar_mul(qT[:, ib, bass.ts(nb, NB)], pq[:, :],
                                    sq_sb[:, ib:ib + 1])

    def proj_k(ib):
        for nb in range(M // NB):
            pk = PJ.tile([128, NB], F32, tag="pj")
            for kb in range(8):
                nc.tensor.matmul(pk[:, :], wk[:, kb, bass.ts(ib, 128)],
                                 cT[:, kb, bass.ts(nb, NB)],
                                 start=(kb == 0), stop=False)
            nc.tensor.matmul(pk[:, :], bk[:, bass.ts(ib, 128)],
                             low[:, bass.ts(nb, NB)], start=False, stop=True)
            nc.vector.tensor_scalar_mul(kT[:, ib, bass.ts(nb, NB)], pk[:, :],
                                        sk_sb[:, ib:ib + 1])

    def v_chunk(mb):
        pv = PJ.tile([128, NB], F32, tag="pj")
        for kb in range(8):
            nc.tensor.matmul(pv[:, 0:IS], cT[:, kb, bass.ts(mb, 128)],
                             wv[:, kb, :], start=(kb == 0), stop=False)
        nc.tensor.matmul(pv[:, 0:IS], low[:, bass.ts(mb, 128)], bv[:],
                         start=False, stop=True)
        nc.vector.tensor_mul(
            out=vA[:, mb, :, 0:DH],
            in0=pv[:, 0:IS].rearrange("p (h e) -> p h e", h=NHEADS),
            in1=svb[:])

    def attention_nb(p, nb, emit_v=False):
        po0 = PO.tile([DH + 1, NB], F32, tag="po")
        po1 = PO.tile([DH + 1, NB], F32, tag="po")
        pos = (po0, po1)
        for mb in range(N_MB):
            if emit_v:
                v_chunk(mb)
            ps = PS.tile([128, 2, NB], F32, tag="ps")
            nc.tensor.matmul(ps[:, 0, :], kT[0:64, p, bass.ts(mb, 128)],
                             qT[0:64, p, bass.ts(nb, NB)],
                             start=True, stop=True, tile_position=(0, 0))
            nc.tensor.matmul(ps[:, 1, :], kT[64:128, p, bass.ts(mb, 128)],
                             qT[64:128, p, bass.ts(nb, NB)],
                             start=True, stop=True, tile_position=(64, 0))
            e = WK.tile([128, 2, NB], BF16, tag="e")
            nc.scalar.activation(e[:], ps[:], AF.Exp, scale=SCALE)
            for j in range(2):
                nc.tensor.matmul(pos[j][:, :], vA[:, mb, 2 * p + j, :],
                                 e[:, j, :], start=(mb == 0), stop=(mb == N_MB - 1),
                                 skip_group_check=True)
        # normalize: out[dh, n] *= 1/denom[n], per head
        for j in range(2):
            po = pos[j]
            den = WK.tile([1, NB], BF16, tag="den")
            nc.vector.tensor_copy(den[:], po[DH:DH + 1, :])
            bc = PJ.tile([128, NB], F32, tag="pj")
            nc.tensor.matmul(bc[0:DH, :], ones64[:], den[:],
                             start=True, stop=True)
            bcs = WK.tile([64, NB], F32, tag="bcs")
            nc.vector.reciprocal(bcs[:], bc[0:DH, :])
            if j == 0:
                # even head of the pair lands on partitions 0:64 directly
                nc.vector.tensor_mul(out=oT[0:64, p, bass.ts(nb, NB)],
                                     in0=po[0:DH, :], in1=bcs[:])
            else:
                # odd head: normalize to a temp, shift to partitions 64:128
                # via identity matmul (col tile_position), copy back aligned
                o4h = WK.tile([64, NB], BF16, tag="o4h")
                nc.vector.tensor_mul(out=o4h[:], in0=po[0:DH, :], in1=bcs[:])
                psh = PJ.tile([128, NB], F32, tag="pj")
                nc.tensor.matmul(psh[64:128, :], ident[:], o4h[:],
                                 start=True, stop=True, tile_position=(0, 64))
                nc.vector.tensor_copy(oT[64:128, p, bass.ts(nb, NB)],
                                      psh[64:128, :])

    def to_out(db, nb):
        pf = PJ.tile([128, NB], F32, tag="pj")
        for kb in range(2):
            nc.tensor.matmul(pf[:, :], wo[:, kb, bass.ts(db, 128)],
                             oT[:, kb, bass.ts(nb, NB)],
                             start=(kb == 0), stop=(kb == 1))
        f = WK.tile([128, NB], F32, tag="fout")
        nc.any.tensor_scalar_mul(f[:], pf[:, :], so_sb[:, db:db + 1])
        nc.sync.dma_start(
            d["po"][bass.ts(db, 128), bass.ts(nb, NB)], f[:])

    proj_k(0)
    proj_q_chunk(0, 0)
    # attention pair 0 starts as early as possible: its v-projection chunks
    # are emitted inline with the first nb so attnv never waits long, and
    # later projections fill PE while ScalarE chews exp
    attention_nb(0, 0, emit_v=True)
    proj_q_chunk(0, 1)
    attention_nb(0, 1)
    proj_k(1)
    proj_q_chunk(0, 2)
    attention_nb(0, 2)
    for nb in range(N_NB):
        proj_q_chunk(1, nb)
    proj_q_chunk(0, 3)
    attention_nb(0, 3)
    for nb in range(N_NB):
        attention_nb(1, nb)
        for db in range(8):
            to_out(db, nb)

    # ---- reduce partials across the batch group, pack 12-bit output ----
    # per output row (d-channel): s = absmax/2046, y = trunc(val/s) + 2048 in
    # [1, 4095]; ship y's high byte hi = (y - y%16)/16 (uint8), packed low
    # nibbles lo[2j] + 16*lo[2j+1] (uint8), and the row absmax (f32). All
    # arithmetic is exact integer-valued f32/i16 (no shifts), so the split is
    # independent of the convert rounding mode. Host: q = 16*hi + lo - 2048.
    nc.gpsimd.collective_compute(
        "ReduceScatter", mybir.AluOpType.add, replica_groups=G4,
        ins=[d["po"][:]], outs=[d["ro"][:]])
    ALU = mybir.AluOpType
    I16 = mybir.dt.int16
    U8 = mybir.dt.uint8
    CN = 512
    with tc.tile_pool(name="fin", bufs=1) as FIN:
        for i in range(2):
            rows = slice(i * 128, (i + 1) * 128)
            amp = FIN.tile([128, 4], F32, tag="amp")
            for c in range(4):
                rf = FIN.tile([128, CN], F32, tag="rf")
                nc.sync.dma_start(rf[:], d["ro"][rows, bass.ts(c, CN)])
                nc.vector.tensor_reduce(
                    amp[:, c:c + 1], rf[:], axis=mybir.AxisListType.X,
                    op=ALU.max, apply_absolute_value=True)
            am = FIN.tile([128, 1], F32, tag="am")
            nc.vector.tensor_reduce(am[:], amp[:], axis=mybir.AxisListType.X,
                                    op=ALU.max)
            am2 = FIN.tile([128, 1], F32, tag="am2")
            nc.vector.tensor_scalar_max(am2[:], am[:], 1e-30)
            r1 = FIN.tile([128, 1], F32, tag="r1")
            nc.vector.reciprocal(r1[:], am2[:])
            rs = FIN.tile([128, 1], F32, tag="rs")
            nc.vector.tensor_scalar_mul(rs[:], r1[:], 2046.0)
            nc.sync.dma_start(
                d["outp"][rows, 3072:3076].bitcast(F32), am2[:])
            for c in range(4):
                rf = FIN.tile([128, CN], F32, tag="rf")
                nc.sync.dma_start(rf[:], d["ro"][rows, bass.ts(c, CN)])
                yq = FIN.tile([128, CN], I16, tag="yq")
                nc.vector.tensor_scalar(yq[:], rf[:], rs[:], 2048.0,
                                        op0=ALU.mult, op1=ALU.add)
                lo = FIN.tile([128, CN], I16, tag="lo")
                nc.vector.tensor_scalar(lo[:], yq[:], 15, None,
                                        op0=ALU.bitwise_and)
                dd = FIN.tile([128, CN], I16, tag="dd")
                nc.vector.tensor_tensor(out=dd[:], in0=yq[:], in1=lo[:],
                                        op=ALU.subtract)
                df = FIN.tile([128, CN], F32, tag="df")
                nc.vector.tensor_copy(df[:], dd[:])
                hf = FIN.tile([128, CN], F32, tag="hf")
                nc.vector.tensor_scalar_mul(hf[:], df[:], 0.0625)
                h8 = FIN.tile([128, CN], U8, tag="h8")
                nc.vector.tensor_copy(h8[:], hf[:])
                nc.sync.dma_start(d["outp"][rows, bass.ts(c, CN)], h8[:])
                lof = FIN.tile([128, CN], F32, tag="lof")
                nc.vector.tensor_copy(lof[:], lo[:])
                lov = lof.rearrange("p (j two) -> p j two", two=2)
                l16 = FIN.tile([128, CN // 2], F32, tag="l16")
                nc.vector.tensor_scalar_mul(l16[:], lov[:, :, 1], 16.0)
                pkf = FIN.tile([128, CN // 2], F32, tag="pkf")
                nc.vector.tensor_tensor(out=pkf[:], in0=lov[:, :, 0],
                                        in1=l16[:], op=ALU.add)
                pk = FIN.tile([128, CN // 2], U8, tag="pk")
                nc.vector.tensor_copy(pk[:], pkf[:])
                nc.sync.dma_start(
                    d["outp"][rows, N + c * (CN // 2):N + (c + 1) * (CN // 2)],
                    pk[:])

    ctx.close()


def build_nc():
    from concourse import bacc
    nc = bacc.Bacc(None, target_bir_lowering=False)
    d = {
        # per-core I/O; blob8 = [c^T slab, int8 weight pair, x^T slab,
        # then raw bytes of abT/bkT0/b0vT (bf16) and scl (f32)]
        "blob8": nc.dram_tensor("blob8", [BLOB_BYTES], I8, kind="ExternalInput"),
        # single packed output row: [hi bytes (2048), lo nibbles (1024),
        # row absmax f32 (4)] = 3076 B
        "outp": nc.dram_tensor("outp", [IS, N + N // 2 + 4], mybir.dt.uint8,
                               kind="ExternalOutput"),
        # internal DRAM (collective bounce + gathered + partials)
        "cb8": nc.dram_tensor("cb8", [D, SS], I8),
        "wb8": nc.dram_tensor("wb8", [2, 256, D], I8),
        "xb8": nc.dram_tensor("xb8", [D, SS], I8),
        "cg8": nc.dram_tensor("cg8", [4, D, SS], I8),
        "wg8": nc.dram_tensor("wg8", [4, 256, D], I8),
        "xg8": nc.dram_tensor("xg8", [4, D, SS], I8),
        "po": nc.dram_tensor("po", [D, N], F32),
        "ro": nc.dram_tensor("ro", [IS, N], F32),
    }
    with tile.TileContext(nc) as tc:
        _emit(tc, nc, d)
    nc.compile()
    return nc


def get_nc():
    if "nc" not in _CACHE:
        _CACHE["nc"] = build_nc()
    return _CACHE["nc"]


def _get_runner():
    """Build (once) the cached shard_map jit over the bass custom call, plus a
    device-side zeros generator for the donated output buffers."""
    if "runner" in _CACHE:
        return _CACHE["runner"]
    import jax
    import jax.numpy as jnp
    from jax.experimental.shard_map import shard_map
    from jax.sharding import Mesh, NamedSharding, PartitionSpec
    from concourse.bass2jax import (
        _bass_exec_p, install_neuronx_cc_hook, partition_id_tensor)

    nc = get_nc()
    install_neuronx_cc_hook()
    assert nc.dbg_addr is None
    partition_name = (nc.partition_id_tensor.name
                      if nc.partition_id_tensor else None)

    in_names, out_names, out_avals = [], [], []
    for alloc in nc.m.functions[0].allocations:
        if not isinstance(alloc, mybir.MemoryLocationSet):
            continue
        name = alloc.memorylocations[0].name
        if alloc.kind == "ExternalInput":
            if name != partition_name:
                in_names.append(name)
        elif alloc.kind == "ExternalOutput":
            out_names.append(name)
            out_avals.append(jax.core.ShapedArray(
                tuple(alloc.tensor_shape), mybir.dt.np(alloc.dtype)))
    n_params = len(in_names)
    all_names = list(in_names) + list(out_names)
    if partition_name is not None:
        all_names.append(partition_name)
    all_names = tuple(all_names)

    def _body(*args):
        operands = list(args)
        if partition_name is not None:
            operands.append(partition_id_tensor())
        outs = _bass_exec_p.bind(
            *operands,
            out_avals=tuple(out_avals),
            in_names=all_names,
            out_names=tuple(out_names),
            lowering_input_output_aliases=(),
            sim_require_finite=True,
            sim_require_nnan=True,
            nc=nc,
        )
        return tuple(outs)

    n_cores = 8
    devices = jax.devices()[:n_cores]
    assert len(devices) == n_cores
    mesh = Mesh(np.asarray(devices), ("core",))
    in_specs = (PartitionSpec("core"),) * (n_params + len(out_names))
    out_specs = (PartitionSpec("core"),) * len(out_names)
    donate = tuple(range(n_params, n_params + len(out_names)))
    gsh = NamedSharding(mesh, PartitionSpec("core"))

    fn = jax.jit(
        shard_map(_body, mesh=mesh, in_specs=in_specs, out_specs=out_specs,
                  check_rep=False),
        donate_argnums=donate, keep_unused=True)

    zspecs = [(tuple((n_cores * a.shape[0], *a.shape[1:])), a.dtype)
              for a in out_avals]
    mkzeros = jax.jit(
        lambda: tuple(jnp.zeros(s, dt) for s, dt in zspecs),
        out_shardings=tuple(gsh for _ in zspecs))

    runner = {
        "fn": fn, "mkzeros": mkzeros, "in_names": in_names,
        "out_names": out_names, "out_avals": out_avals, "n_cores": n_cores,
    }
    _CACHE["runner"] = runner
    return runner


def run_in_maps(in_maps):
    """Execute the kernel on 8 cores given per-core input dicts; returns
    per-core output dicts (the hot path timed by test.py)."""
    r = _get_runner()
    n_cores = r["n_cores"]

    def _concat(name):
        arrs = [np.asarray(m[name]) for m in in_maps]
        base = arrs[0].base
        if (base is not None and base.ndim == 2
                and all(a.base is base for a in arrs)
                and all(np.shares_memory(a, base[i]) for i, a in enumerate(arrs))):
            return base.reshape(-1)  # rows of one prebuilt array, no copy
        return np.concatenate(arrs, axis=0)

    concat_in = [_concat(name) for name in r["in_names"]]
    # donated output buffers: reuse the previous call's device outputs (the
    # kernel overwrites every element), falling back to on-device zeros on
    # the first call — no host upload of output buffers either way
    donated = _CACHE.pop("prev_out", None) or r["mkzeros"]()
    out_arrs = r["fn"](*concat_in, *donated)
    _CACHE["prev_out"] = out_arrs
    results = []
    np_outs = [np.asarray(a) for a in out_arrs]
    for c in range(n_cores):
        results.append({
            name: np_outs[i].reshape(n_cores, *r["out_avals"][i].shape)[c]
            for i, name in enumerate(r["out_names"])
        })
    return results


def _qi8(w, axis):
    s = (np.abs(w).max(axis=axis, keepdims=True) / 127.0).astype(np.float32)
    q32 = w / s
    np.rint(q32, out=q32)
    np.clip(q32, -127, 127, out=q32)
    return q32.astype(np.int8), s


def make_in_maps(x, context, task_idx, Wq, Wk, Wv, Ak, Bk, Av, Bv, Wo):
    bf = ml_dtypes.bfloat16

    def pre(w):   # [1024, 256] weight^T -> [256, 1024] partition-major slab
        return w.reshape(8, 128, IS).transpose(1, 0, 2).reshape(256, D)

    def preo(w):  # [256, 1024] Wo^T slice -> [256, 1024] slab
        return w.reshape(2, 128, D).transpose(1, 0, 2).reshape(256, D)

    Wq_i, sq = _qi8(np.asarray(Wq), -1)
    Wk_i, sk = _qi8(np.asarray(Wk), -1)
    Wv_i, sv = _qi8(np.asarray(Wv), -1)
    Wo_i, so = _qi8(np.asarray(Wo), -1)
    xq, cq, sxs, scs = [], [], [], []
    for b in range(2):
        x_i, sx = _qi8(np.asarray(x)[b], 0)
        c_i, sc = _qi8(np.asarray(context)[b], 0)
        xq.append(np.ascontiguousarray(x_i.T))   # [1024, 2048] int8
        cq.append(np.ascontiguousarray(c_i.T))
        sxs.append(sx[0])
        scs.append(sc[0])

    # build all per-core blobs as rows of one array so run_in_maps can pass
    # the base straight to the jit without re-concatenating
    stacked = np.empty((8, BLOB_BYTES), np.int8)
    in_maps = []
    for dev in range(8):
        b, g = dev // 4, dev % 4
        isl = slice(IS * g, IS * g + IS)
        t = int(task_idx[b])
        z16 = np.zeros((16, IS), np.float32)
        if b == 0:
            wi8 = np.stack([pre(Wq_i[isl].T), pre(Wk_i[isl].T)])
        else:
            wi8 = np.stack([pre(Wv_i[isl].T), preo(Wo_i[:, isl].T)])
        scl = np.concatenate([
            sq[isl, 0], sk[isl, 0], sv[isl, 0], so[:, 0], sxs[b], scs[b],
        ]).astype(np.float32)
        abT = np.ascontiguousarray(
            np.concatenate([Ak[t].T, Av[t].T], axis=1).astype(bf))
        bkT0 = np.ascontiguousarray(np.concatenate(
            [(Bk[t][isl] / sk[isl]).T, z16], axis=0).astype(bf))
        b0vT = np.ascontiguousarray(np.concatenate(
            [z16, (Bv[t][isl] / sv[isl]).T], axis=0).astype(bf))
        blob8 = stacked[dev]
        blob8[0:REG] = cq[b][:, SS * g:SS * (g + 1)].reshape(-1).view(np.int8)
        blob8[REG:2 * REG] = wi8.reshape(-1).view(np.int8)
        blob8[2 * REG:3 * REG] = (
            xq[b][:, SS * g:SS * (g + 1)].reshape(-1).view(np.int8))
        blob8[AB_OFF:BK_OFF] = abT.view(np.int8).reshape(-1)
        blob8[BK_OFF:BV_OFF] = bkT0.view(np.int8).reshape(-1)
        blob8[BV_OFF:SCL_OFF] = b0vT.view(np.int8).reshape(-1)
        blob8[SCL_OFF:BLOB_BYTES] = scl.view(np.int8).reshape(-1)
        in_maps.append({"blob8": blob8})
    return in_maps


def combine(results, bo):
    B = 2
    out = np.empty((B, N, D), np.float32)
    q = np.empty((IS, N), np.int16)
    for b in range(B):
        for g in range(4):
            outp = results[4 * b + g]["outp"]
            oh = outp[:, 0:N].astype(np.int16)
            ol = outp[:, N:N + N // 2]
            q[:, 0::2] = (oh[:, 0::2] << 4) | (ol & 15)
            q[:, 1::2] = (oh[:, 1::2] << 4) | (ol >> 4)
            q -= 2048
            osc = np.ascontiguousarray(outp[:, 3072:3076]).view(np.float32)[:, 0]
            rowsc = (osc / 2046.0).astype(np.float32)
            out[b, :, IS * g:IS * (g + 1)] = (q * rowsc[:, None]).T
    out += np.asarray(bo).astype(np.float32)
    return out


def kernel(x, context, mask, task_idx, Wq, Wk, Wv, Ak, Bk, Av, Bv, Wo, bo):
    # mask is all-ones per the input spec; softmax ignores it.
    args = [np.asarray(a) for a in
            (x, context, task_idx, Wq, Wk, Wv, Ak, Bk, Av, Bv, Wo)]
    in_maps = make_in_maps(*args)
    results = run_in_maps(in_maps)
    return combine(results, np.asarray(bo))
